# revision 85
# baseline (speedup 1.0000x reference)
"""Linformer multi-head attention on 8 Trainium2 NeuronCores.

Sharding: data-parallel over batch (BATCH=8 -> 1 batch element per core).
Per core (x transposed on host to xT [512, 4096]):
  A1: QT = wq.T-chunks @ xT (stored [512, 4096] as 4 tiles);
      K[s] = xT-chunks.T @ wk per 128-row tile;
      kpT accumulated in 4 resident PSUM banks over the whole sequence via
      head-PAIR matmuls: weights = [k_{2i} | k_{2i+1}] (128 wide), moving
      [E_{2i} | E_{2i+1}] (512 cols) -> diagonal blocks are the real
      k_projT pair; off-diagonal quadrants are discarded.
  A2: same for V / vpT with F (second pass re-streams xT so kp and vp each
      get 4 PSUM banks without colliding); vext = natural-layout v_proj
      (+ ones column for the softmax denominator) built via PE transposes.
  C:  scoresT [256, n] per head = kpT-chunks.T @ QT; exp on scalar engine;
      PV with ones column gives outT (rows 0..63) + denom (row 64); the
      pair's raw denom rows (partitions 0/64 of a [65,512] bf16 scratch)
      are broadcast with a [65,128] 0/1-selector matmul, inverted by one
      reciprocal_approx_fast over the broadcast [128,512], then
      oT = outT * rec;  y = oT.T-chunks @ w_out + bias via a rank-1
      ones-column matmul folded into the same PSUM accumulation.
      Software-pipelined: each pair's normalize chain issues mid-way
      through the next pair's block, and tile j's output dense issues
      inside tile j+1 (with a dm-split epilogue for the last tile).

Engine placement: exp + one den copy + half the fin copies on Scalar;
casts, recip, muls, the other halves on Vector. bf16 compute, fp32 PSUM.
"""

import numpy as np
import ml_dtypes

BATCH, SEQ, DM = 8, 4096, 512
NH, DH, R = 8, 64, 256
NCORES = 8
NT = SEQ // 512  # 8 big n-tiles of 512 rows

_built = {}


def prep_x(x_i):
    """Host-side prep of one batch element: bf16 + transpose to [DM, SEQ]."""
    return np.ascontiguousarray(
        np.asarray(x_i, dtype=ml_dtypes.bfloat16).T
    )


def prep_ef(e):
    """Host-side prep of E/F [H, SEQ, R] -> bf16 [j, p, s, h, r] so each
    per-j-tile DMA is one contiguous 2-D transfer."""
    e = np.asarray(e, dtype=ml_dtypes.bfloat16)
    e = e.reshape(NH, NT, 4, 128, R).transpose(1, 3, 2, 0, 4)
    return np.ascontiguousarray(e)


def _build():
    """Build the Bass module (once per process)."""
    if "nc" in _built:
        return _built["nc"]

    from contextlib import ExitStack

    import concourse.bass as bass
    import concourse.bacc as bacc
    import concourse.mybir as mybir
    import concourse.tile as tile
    from concourse.masks import make_identity

    f32 = mybir.dt.float32
    cdt = mybir.dt.bfloat16

    nc = bacc.Bacc("TRN2", target_bir_lowering=False, debug=False)

    x_d = nc.dram_tensor("x", [DM, SEQ], cdt, kind="ExternalInput").ap()
    wq_d = nc.dram_tensor("wq", [DM, DM], cdt, kind="ExternalInput").ap()
    wk_d = nc.dram_tensor("wk", [DM, DM], cdt, kind="ExternalInput").ap()
    wv_d = nc.dram_tensor("wv", [DM, DM], cdt, kind="ExternalInput").ap()
    e_d = nc.dram_tensor("E", [NT, 128, 4, NH, R], cdt, kind="ExternalInput").ap()
    f_d = nc.dram_tensor("F", [NT, 128, 4, NH, R], cdt, kind="ExternalInput").ap()
    wo_d = nc.dram_tensor("w_out", [DM, DM], cdt, kind="ExternalInput").ap()
    b_d = nc.dram_tensor("b_out", [DM], f32, kind="ExternalInput").ap()
    y_d = nc.dram_tensor("y", [SEQ, DM], f32, kind="ExternalOutput").ap()

    with tile.TileContext(nc) as tc, ExitStack() as ctx:
        singles = ctx.enter_context(tc.tile_pool(name="singles", bufs=1))

        ident = singles.tile([128, 128], cdt)
        make_identity(nc, ident)
        # 0/1 selector [128, 128]: rows 0 and 64 carry the two denom rows
        # (compute-engine partition bases must be multiples of 32, so the
        # pair's denominators live at partitions 0 and 64, not 0 and 1);
        # full 128-row contraction so the PE keeps its usual weight width;
        # out[d, n] = den[64 * (d // 64), n]
        sel65 = singles.tile([128, 128], cdt)
        nc.vector.memset(sel65, 0.0)
        nc.vector.memset(sel65[0:1, 0:64], 1.0)
        nc.vector.memset(sel65[64:65, 64:128], 1.0)
        # bf16 denominator scratch (ping-pong by head-pair parity);
        # unwritten rows stay 1.0 forever (they hit zero selector rows)
        den_bf = [singles.tile([128, 512], cdt, name=f"den{i}") for i in range(2)]
        for t in den_bf:
            nc.vector.memset(t, 1.0)
        # rank-1 bias trick: ones column [1, 128] so out[d, m] += bias[m]
        ones_col = singles.tile([1, 128], cdt)
        nc.vector.memset(ones_col, 1.0)
        bias_bf = singles.tile([1, DM], cdt)
        # weights as [128, dk, 512]: chunk dk holds rows dk*128..+128;
        # wq/wk/wv lead the DMA queue so phase A starts ASAP; wo and the
        # bias broadcast are DMA'd at the top of phase C instead
        w_sb = {}
        for name, d in (("wq", wq_d), ("wk", wk_d), ("wv", wv_d), ("wo", wo_d)):
            t = singles.tile([128, 4, DM], cdt, name=f"w_{name}")
            if name == "wq":
                nc.sync.dma_start(out=t, in_=d.rearrange("(dk p) m -> p dk m", p=128))
            w_sb[name] = t
        bias_bc128 = singles.tile([128, DM], f32)

        # QT global [512, 4096] as 4 tiles [128, 4096]; tile t = heads 2t,2t+1
        qt_g = [singles.tile([128, SEQ], cdt, tag=f"qt{t}", name=f"qt{t}") for t in range(4)]
        # pair-packed projections, 4 tiles [128, 512] each:
        #   pair i = heads (2i, 2i+1); head 2i at [0:64, 0:256],
        #   head 2i+1 at [64:128, 256:512]
        kp_sb = [singles.tile([128, 2 * R], cdt, tag=f"kp{i}", name=f"kpT{i}") for i in range(4)]
        vp_sb = [singles.tile([128, 2 * R], cdt, tag=f"vp{i}", name=f"vpT{i}") for i in range(4)]

        def hslice(sb, h):
            """[64, 256] packed slice for head h."""
            i, off = h // 2, h % 2
            return sb[i][off * 64 : (off + 1) * 64, off * R : (off + 1) * R]

        # v_proj natural chunks + ones column: [128, h, rc, 65]
        vext = singles.tile([128, NH, 2, 65], cdt)

        x_r = x_d.rearrange("(dk p) n -> p dk n", p=128)
        e_r = e_d.rearrange("j p s h r -> p j s h r")
        f_r = f_d.rearrange("j p s h r -> p j s h r")

        # ---------------- Pass A: QT, K, kpT, then V, vpT ----------------
        # One scope: A2's V chains rotate into A1's QT banks and the vp
        # accumulators rotate (same tags) into the kp banks, so there is no
        # pool-teardown barrier between the two sweeps over x.
        with (
            tc.tile_pool(name="p_x1", bufs=3) as p_x,
            tc.tile_pool(name="p_e", bufs=3) as p_e,
            tc.tile_pool(name="p_k", bufs=6) as p_k,
            tc.tile_pool(name="ps_qt", bufs=2, space="PSUM") as ps_qt,
            tc.tile_pool(name="ps_k", bufs=2, space="PSUM") as ps_k,
            tc.tile_pool(name="ps_kp", bufs=1, space="PSUM") as ps_kp,
        ):
            kp_ps = [
                ps_kp.tile([128, 2 * R], f32, tag=f"acc{i}", name=f"kp_ps{i}")
                for i in range(4)
            ]
            # x tiles are prefetched one j ahead so each 0.5 MB x transfer
            # queues BEFORE the previous tile's 2 MB E transfer
            xts = {}

            def load_x(jj, tag):
                # gpsimd's own DMA queue: x transfers run in parallel with
                # the weight/E/F bulk on the sync queue (gpsimd is idle)
                t = p_x.tile([128, 4, 512], cdt, tag="xt", name=f"xt{tag}_{jj}")
                nc.gpsimd.dma_start(out=t, in_=x_r[:, :, jj * 512 : (jj + 1) * 512])
                return t

            xts[0] = load_x(0, "a")
            for j in range(NT):
                xt = xts.pop(j)
                if j == 0:
                    # behind wq and x0 in the queue: ready before first use
                    for name, d in (("wk", wk_d), ("wv", wv_d)):
                        nc.sync.dma_start(
                            out=w_sb[name],
                            in_=d.rearrange("(dk p) m -> p dk m", p=128),
                        )
                if j + 1 < NT:
                    xts[j + 1] = load_x(j + 1, "a")

                # QT_j[dq] [128, 512] = sum_dk wq[dk, dq-chunk].T @ xT[dk]
                for dq in range(4):
                    pq = ps_qt.tile([128, 512], f32, tag="pq")
                    for dk in range(4):
                        nc.tensor.matmul(
                            pq,
                            w_sb["wq"][:, dk, dq * 128 : (dq + 1) * 128],
                            xt[:, dk, :],
                            start=(dk == 0),
                            stop=(dk == 3),
                        )
                    nc.vector.tensor_copy(qt_g[dq][:, j * 512 : (j + 1) * 512], pq)

                # all K chains first, then all projection groups: the scalar
                # PSUM->SBUF cast of chunk s completes while later K chunks
                # stream, so the kp matmuls never stall the PE
                e_t = p_e.tile([128, 4, NH, R], cdt, tag="e", name=f"e_{j}")
                nc.sync.dma_start(out=e_t, in_=e_r[:, j, :, :, :])
                k_sbs = []
                for s in range(4):
                    ti = j * 4 + s
                    pk = ps_k.tile([128, 512], f32, tag="pk")
                    for dk in range(4):
                        nc.tensor.matmul(
                            pk,
                            xt[:, dk, s * 128 : (s + 1) * 128],
                            w_sb["wk"][:, dk, :],
                            start=(dk == 0),
                            stop=(dk == 3),
                        )
                    k_sb = p_k.tile([128, 512], cdt, tag="k", name=f"k_{ti}")
                    nc.scalar.copy(k_sb, pk)
                    k_sbs.append(k_sb)

                for s in range(4):
                    ti = j * 4 + s
                    for i in range(4):
                        nc.tensor.matmul(
                            kp_ps[i],
                            k_sbs[s][:, i * 128 : (i + 1) * 128],
                            e_t[:, s, 2 * i : 2 * i + 2, :],
                            start=(ti == 0),
                            stop=(ti == 31),
                        )
            for i in range(4):
                nc.vector.tensor_copy(kp_sb[i], kp_ps[i])

            # ---- second sweep: V, vpT (same pools, rotated buffers) ----
            vp_ps = [
                ps_kp.tile([128, 2 * R], f32, tag=f"acc{i}", name=f"vp_ps{i}")
                for i in range(4)
            ]
            xts[0] = load_x(0, "b")
            for j in range(NT):
                xt = xts.pop(j)
                if j + 1 < NT:
                    xts[j + 1] = load_x(j + 1, "b")

                f_t = p_e.tile([128, 4, NH, R], cdt, tag="e", name=f"f_{j}")
                nc.sync.dma_start(out=f_t, in_=f_r[:, j, :, :, :])
                v_sbs = []
                for s in range(4):
                    ti = j * 4 + s
                    pv = ps_qt.tile([128, 512], f32, tag="pq")
                    for dk in range(4):
                        nc.tensor.matmul(
                            pv,
                            xt[:, dk, s * 128 : (s + 1) * 128],
                            w_sb["wv"][:, dk, :],
                            start=(dk == 0),
                            stop=(dk == 3),
                        )
                    v_sb = p_k.tile([128, 512], cdt, tag="k", name=f"v_{ti}")
                    nc.scalar.copy(v_sb, pv)
                    v_sbs.append(v_sb)

                for s in range(4):
                    ti = j * 4 + s
                    for i in range(4):
                        nc.tensor.matmul(
                            vp_ps[i],
                            v_sbs[s][:, i * 128 : (i + 1) * 128],
                            f_t[:, s, 2 * i : 2 * i + 2, :],
                            start=(ti == 0),
                            stop=(ti == 31),
                        )
            # vext: per pair, cast vpT then transpose that pair's heads to
            # natural layout + ones col, so the PE starts transposing as
            # soon as the first cast retires (ps_k's banks are free once
            # the last V cast retires)
            for i in range(4):
                nc.vector.tensor_copy(vp_sb[i], vp_ps[i])
                for h in (2 * i, 2 * i + 1):
                    off = h % 2
                    pt_f = ps_k.tile([128, 512], f32, tag="pk", name=f"ptf_{h}")
                    pt = pt_f.bitcast(cdt)[:, 0:128]
                    for rc in range(2):
                        nc.tensor.transpose(
                            pt[:, rc * 64 : (rc + 1) * 64],
                            hslice(vp_sb, h)[:, rc * 128 : (rc + 1) * 128],
                            ident[off * 64 : off * 64 + 64, off * 64 : off * 64 + 64],
                        )
                    for rc in range(2):
                        nc.vector.tensor_copy(
                            vext[:, h, rc, 0:64], pt[:, rc * 64 : (rc + 1) * 64]
                        )
                    nc.vector.memset(vext[:, h, :, 64:65], 1.0)

        # ---------------- Phase C: attention + output dense ----------------
        # Software-pipelined: the bc/normalize of head-pair hp issues after
        # the scores/PV of hp+1 (hiding the denominator chain latency), and
        # the output dense of tile j issues inside tile j+1's first block.
        y_r = y_d.rearrange("(j s p) m -> p j s m", s=4, p=128)
        with (
            tc.tile_pool(name="p_at", bufs=6) as p_at,
            tc.tile_pool(name="p_bc", bufs=3) as p_bc,
            tc.tile_pool(name="p_ot", bufs=8) as p_ot,
            tc.tile_pool(name="p_fin", bufs=2) as p_fin,
            # pool order matters: ps_out grabs the banks still held by the
            # A-scope's QT/K pools (PV starts late — plenty of slack) while
            # ps_scfin gets the accumulator banks, which free right after
            # the vp casts — so the first scores never wait on vext
            tc.tile_pool(name="ps_out", bufs=4, space="PSUM") as ps_out,
            tc.tile_pool(name="ps_scfin", bufs=3, space="PSUM") as ps_scfin,
            tc.tile_pool(name="ps_bc", bufs=1, space="PSUM") as ps_bc,
        ):
            nc.sync.dma_start(
                out=w_sb["wo"], in_=wo_d.rearrange("(dk p) m -> p dk m", p=128)
            )
            b_bc_ap = bass.AP(
                tensor=b_d.tensor, offset=b_d.offset, ap=[[0, 128]] + list(b_d.ap)
            )
            nc.sync.dma_start(out=bias_bc128, in_=b_bc_ap)
            nc.vector.tensor_copy(bias_bf, bias_bc128[0:1, :])

            pend = None  # (oT list, hp, outps) awaiting bc+normalize
            pend_fin = None  # oT list awaiting output dense

            def attn_block(j, hp, oT):
                """scores + exp for both heads, then PV for both heads —
                each PV's exp input gets >=2 matmuls of cover, so the PE
                never waits on the scalar engine. The previous pair's bc
                flush is issued AFTER this block: its den rows then beat
                the recip/muls into the Vector queue, and the bc matmul
                gets a full block of PE cover."""
                den = den_bf[hp % 2]
                ats, outps = [], []
                for hh in range(2):
                    h = hp * 2 + hh
                    qrow = qt_g[h // 2][
                        (h % 2) * 64 : (h % 2) * 64 + 64, j * 512 : (j + 1) * 512
                    ]
                    at = []
                    for rc in range(2):
                        sc = ps_scfin.tile([128, 512], f32, tag="sf")
                        nc.tensor.matmul(
                            sc,
                            hslice(kp_sb, h)[:, rc * 128 : (rc + 1) * 128],
                            qrow,
                            start=True,
                            stop=True,
                        )
                        a = p_at.tile([128, 512], cdt, tag="at")
                        nc.scalar.activation(
                            a, sc, mybir.ActivationFunctionType.Exp, scale=0.125
                        )
                        at.append(a)
                    ats.append(at)
                for hh in range(2):
                    h = hp * 2 + hh
                    # PV with ones column: rows 0..63 = outT, row 64 = denom
                    op = ps_out.tile([128, 512], f32, tag="op")
                    for rc in range(2):
                        nc.tensor.matmul(
                            op[0:65, :],
                            vext[:, h, rc, :],
                            ats[hh][rc],
                            start=(rc == 0),
                            stop=(rc == 1),
                        )
                    outps.append(op)
                    if hh == 0:
                        nc.scalar.copy(den[0:1, :], op[64:65, :])
                    else:
                        nc.vector.tensor_copy(den[64:65, :], op[64:65, :])
                return outps

            def flush_bc(oT, hp, outps):
                """selector-broadcast raw denoms, reciprocal, normalize."""
                bc = ps_bc.tile([128, 512], f32, tag="bc")
                nc.tensor.matmul(bc, sel65, den_bf[hp % 2], start=True, stop=True)
                bc_sb = p_bc.tile([128, 512], f32, tag="bcs")
                nc.vector.reciprocal_approx_fast(out=bc_sb, in_=bc)
                for hh in range(2):
                    nc.vector.tensor_mul(
                        oT[hp][hh * 64 : (hh + 1) * 64, :],
                        outps[hh][0:64, :],
                        bc_sb[hh * 64 : (hh + 1) * 64, :],
                    )

            def fin_chunk(fin, j, oT, s, fp=None):
                """one 128-row dense chunk. Even s: rank-1 bias matmul +
                scalar copy; odd s: bias folded into the vector transfer
                as a tensor_add (same cost as the copy it replaces)."""
                odd = s % 2 == 1
                first = 3 if fp is not None else 0
                if fp is None:
                    fp = ps_scfin.tile([128, 512], f32, tag="sf")
                for dm in range(first, 4):
                    nc.tensor.matmul(
                        fp,
                        oT[dm][:, s * 128 : (s + 1) * 128],
                        w_sb["wo"][:, dm, :],
                        start=(dm == 0),
                        stop=(odd and dm == 3),
                    )
                if odd:
                    nc.vector.tensor_add(fin[:, s, :], fp, bias_bc128)
                else:
                    nc.tensor.matmul(fp, ones_col, bias_bf, start=False, stop=True)
                    nc.scalar.copy(fin[:, s, :], fp)
                nc.gpsimd.dma_start(out=y_r[:, j, s, :], in_=fin[:, s, :])

            def flush_fin(j, oT):
                """output dense + bias for tile j (issued during j+1)."""
                fin = p_fin.tile([128, 4, 512], f32, tag="fin", name=f"fin_{j}")
                for s in range(4):
                    fin_chunk(fin, j, oT, s)

            for j in range(NT):
                oT = [
                    p_ot.tile([128, 512], cdt, tag="ot", name=f"oT{j}_{t}")
                    for t in range(4)
                ]
                for hp in range(4):
                    outps = attn_block(j, hp, oT)
                    if pend is not None:
                        flush_bc(*pend)
                    if hp == 0 and pend_fin is not None:
                        flush_fin(j - 1, pend_fin)
                        pend_fin = None
                    pend = (oT, hp, outps)
                pend_fin = oT
            # last tile: overlap the final pair's normalize chain with
            # partial dense chains (oT[dm] only depends on head pair dm,
            # so dm=0..2 can accumulate before hp3's normalize lands)
            oT = pend_fin
            fin = p_fin.tile([128, 4, 512], f32, tag="fin", name="fin_last")
            fps = []
            for s in range(3):
                fp = ps_scfin.tile([128, 512], f32, tag="sf")
                for dm in range(3):
                    nc.tensor.matmul(
                        fp,
                        oT[dm][:, s * 128 : (s + 1) * 128],
                        w_sb["wo"][:, dm, :],
                        start=(dm == 0),
                        stop=False,
                    )
                fps.append(fp)
                if s == 0:
                    flush_bc(*pend)
            for s in range(3):
                fin_chunk(fin, NT - 1, oT, s, fp=fps[s])
            fin_chunk(fin, NT - 1, oT, 3)

    nc.compile()
    _built["nc"] = nc
    return nc


def _runner():
    """Build (once) a cached jitted 8-core executor for the Bass module."""
    if "run" in _built:
        return _built["run"]

    import jax
    import numpy as _np

    import concourse.mybir as mybir
    from concourse import bass2jax

    bass2jax.install_neuronx_cc_hook()
    nc = _build()

    part_name = nc.partition_id_tensor.name if nc.partition_id_tensor else None
    in_names, out_names, out_avals = [], [], []
    for alloc in nc.m.functions[0].allocations:
        if not isinstance(alloc, mybir.MemoryLocationSet):
            continue
        name = alloc.memorylocations[0].name
        if alloc.kind == "ExternalInput":
            if name != part_name:
                in_names.append(name)
        elif alloc.kind == "ExternalOutput":
            out_names.append(name)
            out_avals.append(
                jax.core.ShapedArray(
                    tuple(alloc.tensor_shape), mybir.dt.np(alloc.dtype)
                )
            )
    n_outs = len(out_avals)
    all_in_names = tuple(
        in_names + out_names + ([part_name] if part_name else [])
    )

    from jax.sharding import NamedSharding

    def _body(*args):
        operands = list(args)
        if part_name is not None:
            operands.append(bass2jax.partition_id_tensor())
        outs = bass2jax._bass_exec_p.bind(
            *operands,
            out_avals=tuple(out_avals),
            in_names=all_in_names,
            out_names=tuple(out_names),
            lowering_input_output_aliases=(),
            sim_require_finite=True,
            sim_require_nnan=True,
            nc=nc,
        )
        return tuple(outs)

    devices = jax.devices()[:NCORES]
    mesh = bass2jax.Mesh(_np.asarray(devices), ("core",))
    p_core = bass2jax.PartitionSpec("core")
    p_repl = bass2jax.PartitionSpec()
    in_specs = tuple(p_core if n == "x" else p_repl for n in in_names) + (
        p_core,
    ) * n_outs
    sharded = jax.jit(
        bass2jax.shard_map(
            _body,
            mesh=mesh,
            in_specs=in_specs,
            out_specs=(p_core,) * n_outs,
            check_rep=False,
        ),
        keep_unused=True,
    )
    sh_core = NamedSharding(mesh, p_core)
    sh_repl = NamedSharding(mesh, p_repl)
    dev_cache = {}
    zero_cache = {}

    def run(in_maps):
        args = []
        for name in in_names:
            if name == "x":
                xc = np.concatenate([np.asarray(m[name]) for m in in_maps], axis=0)
                args.append(jax.device_put(xc, sh_core))
            else:
                a = np.asarray(in_maps[0][name])
                key = (name, a.shape, str(a.dtype), hash(a.tobytes()))
                if key not in dev_cache:
                    dev_cache.clear() if len(dev_cache) > 64 else None
                    dev_cache[key] = jax.device_put(a, sh_repl)
                args.append(dev_cache[key])
        for i, a in enumerate(out_avals):
            if i not in zero_cache:
                zero_cache[i] = jax.device_put(
                    np.zeros((NCORES * a.shape[0], *a.shape[1:]), a.dtype), sh_core
                )
            args.append(zero_cache[i])
        out_arrs = sharded(*args)
        return [
            {
                name: np.asarray(out_arrs[i]).reshape(
                    NCORES, *out_avals[i].shape
                )[c]
                for i, name in enumerate(out_names)
            }
            for c in range(NCORES)
        ]

    _built["run"] = run
    return run


def kernel(x, wq, wk, wv, E, F, w_out, b_out):
    """Full inputs in, full output out. Shards batch across 8 cores."""
    run = _runner()

    np_c = ml_dtypes.bfloat16
    wq_c = np.ascontiguousarray(wq, dtype=np_c)
    wk_c = np.ascontiguousarray(wk, dtype=np_c)
    wv_c = np.ascontiguousarray(wv, dtype=np_c)
    e_c = prep_ef(E)
    f_c = prep_ef(F)
    wo_c = np.ascontiguousarray(w_out, dtype=np_c)
    b_c = np.ascontiguousarray(b_out, dtype=np.float32)

    in_maps = [
        {
            "x": prep_x(x[i]),
            "wq": wq_c,
            "wk": wk_c,
            "wv": wv_c,
            "E": e_c,
            "F": f_c,
            "w_out": wo_c,
            "b_out": b_c,
        }
        for i in range(NCORES)
    ]
    results = run(in_maps)
    return np.stack([results[i]["y"] for i in range(NCORES)], axis=0)


if __name__ == "__main__":
    xs = {
        "x": np.random.randn(BATCH, SEQ, DM).astype(np.float32),
        "wq": np.random.randn(DM, DM).astype(np.float32) * 0.05,
        "wk": np.random.randn(DM, DM).astype(np.float32) * 0.05,
        "wv": np.random.randn(DM, DM).astype(np.float32) * 0.05,
        "E": np.random.randn(NH, SEQ, R).astype(np.float32) * 0.03,
        "F": np.random.randn(NH, SEQ, R).astype(np.float32) * 0.03,
        "w_out": np.random.randn(DM, DM).astype(np.float32) * 0.05,
        "b_out": np.zeros(DM, np.float32),
    }
    y = kernel(**xs)
    print(y.shape, y.dtype)


# revision 86
# speedup vs baseline: 1.0321x; 1.0321x over previous
"""Linformer multi-head attention on 8 Trainium2 NeuronCores.

Sharding: data-parallel over batch (BATCH=8 -> 1 batch element per core).
Per core (x transposed on host to xT [512, 4096]):
  A1: QT = wq.T-chunks @ xT (stored [512, 4096] as 4 tiles);
      K[s] = xT-chunks.T @ wk per 128-row tile;
      kpT accumulated in 4 resident PSUM banks over the whole sequence via
      head-PAIR matmuls: weights = [k_{2i} | k_{2i+1}] (128 wide), moving
      [E_{2i} | E_{2i+1}] (512 cols) -> diagonal blocks are the real
      k_projT pair; off-diagonal quadrants are discarded.
  A2: same for V / vpT with F (second pass re-streams xT so kp and vp each
      get 4 PSUM banks without colliding); vext = natural-layout v_proj
      (+ ones column for the softmax denominator) built via PE transposes.
  C:  scoresT [256, n] per head = kpT-chunks.T @ QT; exp on scalar engine;
      PV with ones column gives outT (rows 0..63) + denom (row 64); the
      pair's raw denom rows (partitions 0/64 of a [65,512] bf16 scratch)
      are broadcast with a [65,128] 0/1-selector matmul, inverted by one
      reciprocal_approx_fast over the broadcast [128,512], then
      oT = outT * rec;  y = oT.T-chunks @ w_out + bias via a rank-1
      ones-column matmul folded into the same PSUM accumulation.
      Software-pipelined: each pair's normalize chain issues mid-way
      through the next pair's block, and tile j's output dense issues
      inside tile j+1 (with a dm-split epilogue for the last tile).

Engine placement: exp + one den copy + half the fin copies on Scalar;
casts, recip, muls, the other halves on Vector. bf16 compute, fp32 PSUM.
"""

import numpy as np
import ml_dtypes

BATCH, SEQ, DM = 8, 4096, 512
NH, DH, R = 8, 64, 256
NCORES = 8
NT = SEQ // 512  # 8 big n-tiles of 512 rows

_built = {}


def prep_x(x_i):
    """Host-side prep of one batch element: bf16 + transpose to [DM, SEQ]."""
    return np.ascontiguousarray(
        np.asarray(x_i, dtype=ml_dtypes.bfloat16).T
    )


def prep_ef(e):
    """Host-side prep of E/F [H, SEQ, R] -> bf16 [j, p, s, h, r] so each
    per-j-tile DMA is one contiguous 2-D transfer."""
    e = np.asarray(e, dtype=ml_dtypes.bfloat16)
    e = e.reshape(NH, NT, 4, 128, R).transpose(1, 3, 2, 0, 4)
    return np.ascontiguousarray(e)


def _build():
    """Build the Bass module (once per process)."""
    if "nc" in _built:
        return _built["nc"]

    from contextlib import ExitStack

    import concourse.bass as bass
    import concourse.bacc as bacc
    import concourse.mybir as mybir
    import concourse.tile as tile
    from concourse.masks import make_identity

    f32 = mybir.dt.float32
    cdt = mybir.dt.bfloat16

    nc = bacc.Bacc("TRN2", target_bir_lowering=False, debug=False)

    x_d = nc.dram_tensor("x", [DM, SEQ], cdt, kind="ExternalInput").ap()
    wq_d = nc.dram_tensor("wq", [DM, DM], cdt, kind="ExternalInput").ap()
    wk_d = nc.dram_tensor("wk", [DM, DM], cdt, kind="ExternalInput").ap()
    wv_d = nc.dram_tensor("wv", [DM, DM], cdt, kind="ExternalInput").ap()
    e_d = nc.dram_tensor("E", [NT, 128, 4, NH, R], cdt, kind="ExternalInput").ap()
    f_d = nc.dram_tensor("F", [NT, 128, 4, NH, R], cdt, kind="ExternalInput").ap()
    wo_d = nc.dram_tensor("w_out", [DM, DM], cdt, kind="ExternalInput").ap()
    b_d = nc.dram_tensor("b_out", [DM], f32, kind="ExternalInput").ap()
    y_d = nc.dram_tensor("y", [SEQ, DM], f32, kind="ExternalOutput").ap()

    with tile.TileContext(nc) as tc, ExitStack() as ctx:
        singles = ctx.enter_context(tc.tile_pool(name="singles", bufs=1))

        ident = singles.tile([128, 128], cdt)
        make_identity(nc, ident)
        # 0/1 selector [128, 128]: rows 0 and 64 carry the two denom rows
        # (compute-engine partition bases must be multiples of 32, so the
        # pair's denominators live at partitions 0 and 64, not 0 and 1);
        # full 128-row contraction so the PE keeps its usual weight width;
        # out[d, n] = den[64 * (d // 64), n]
        sel65 = singles.tile([128, 128], cdt)
        nc.vector.memset(sel65, 0.0)
        nc.vector.memset(sel65[0:1, 0:64], 1.0)
        nc.vector.memset(sel65[64:65, 64:128], 1.0)
        # bf16 denominator scratch (ping-pong by head-pair parity);
        # unwritten rows stay 1.0 forever (they hit zero selector rows)
        den_bf = [singles.tile([128, 512], cdt, name=f"den{i}") for i in range(2)]
        for t in den_bf:
            nc.vector.memset(t, 1.0)
        # rank-1 bias trick: ones column [1, 128] so out[d, m] += bias[m]
        ones_col = singles.tile([1, 128], cdt)
        nc.vector.memset(ones_col, 1.0)
        bias_bf = singles.tile([1, DM], cdt)
        # weights as [128, dk, 512]: chunk dk holds rows dk*128..+128;
        # wq/wk/wv lead the DMA queue so phase A starts ASAP; wo and the
        # bias broadcast are DMA'd at the top of phase C instead
        w_sb = {}
        for name, d in (("wq", wq_d), ("wk", wk_d), ("wv", wv_d), ("wo", wo_d)):
            t = singles.tile([128, 4, DM], cdt, name=f"w_{name}")
            if name == "wq":
                nc.sync.dma_start(out=t, in_=d.rearrange("(dk p) m -> p dk m", p=128))
            w_sb[name] = t
        bias_bc128 = singles.tile([128, DM], f32)

        # QT global [512, 4096] as 4 tiles [128, 4096]; tile t = heads 2t,2t+1
        qt_g = [singles.tile([128, SEQ], cdt, tag=f"qt{t}", name=f"qt{t}") for t in range(4)]
        # pair-packed projections, 4 tiles [128, 512] each:
        #   pair i = heads (2i, 2i+1); head 2i at [0:64, 0:256],
        #   head 2i+1 at [64:128, 256:512]
        kp_sb = [singles.tile([128, 2 * R], cdt, tag=f"kp{i}", name=f"kpT{i}") for i in range(4)]
        vp_sb = [singles.tile([128, 2 * R], cdt, tag=f"vp{i}", name=f"vpT{i}") for i in range(4)]

        def hslice(sb, h):
            """[64, 256] packed slice for head h."""
            i, off = h // 2, h % 2
            return sb[i][off * 64 : (off + 1) * 64, off * R : (off + 1) * R]

        # v_proj natural chunks + ones column: [128, h, rc, 65]
        vext = singles.tile([128, NH, 2, 65], cdt)

        x_r = x_d.rearrange("(dk p) n -> p dk n", p=128)
        e_r = e_d.rearrange("j p s h r -> p j s h r")
        f_r = f_d.rearrange("j p s h r -> p j s h r")

        # ---------------- Pass A: QT, K, kpT, then V, vpT ----------------
        # One scope: A2's V chains rotate into A1's QT banks and the vp
        # accumulators rotate (same tags) into the kp banks, so there is no
        # pool-teardown barrier between the two sweeps over x.
        with (
            tc.tile_pool(name="p_x1", bufs=3) as p_x,
            tc.tile_pool(name="p_e", bufs=3) as p_e,
            tc.tile_pool(name="p_k", bufs=6) as p_k,
            tc.tile_pool(name="ps_qt", bufs=2, space="PSUM") as ps_qt,
            tc.tile_pool(name="ps_k", bufs=2, space="PSUM") as ps_k,
            tc.tile_pool(name="ps_kp", bufs=1, space="PSUM") as ps_kp,
        ):
            kp_ps = [
                ps_kp.tile([128, 2 * R], f32, tag=f"acc{i}", name=f"kp_ps{i}")
                for i in range(4)
            ]
            # x tiles are prefetched one j ahead so each 0.5 MB x transfer
            # queues BEFORE the previous tile's 2 MB E transfer
            xts = {}

            def load_x(jj, tag):
                # gpsimd's own DMA queue: x transfers run in parallel with
                # the weight/E/F bulk on the sync queue (gpsimd is idle)
                t = p_x.tile([128, 4, 512], cdt, tag="xt", name=f"xt{tag}_{jj}")
                nc.gpsimd.dma_start(out=t, in_=x_r[:, :, jj * 512 : (jj + 1) * 512])
                return t

            xts[0] = load_x(0, "a")
            for j in range(NT):
                xt = xts.pop(j)
                if j == 0:
                    # behind wq in the queue: ready before the K chains
                    nc.sync.dma_start(
                        out=w_sb["wk"],
                        in_=wk_d.rearrange("(dk p) m -> p dk m", p=128),
                    )
                elif j == 2:
                    # wv is first needed in the second sweep, ~100 us away:
                    # keep it out of E0/E1's way in the sync queue
                    nc.sync.dma_start(
                        out=w_sb["wv"],
                        in_=wv_d.rearrange("(dk p) m -> p dk m", p=128),
                    )
                if j + 1 < NT:
                    xts[j + 1] = load_x(j + 1, "a")

                # QT_j[dq] [128, 512] = sum_dk wq[dk, dq-chunk].T @ xT[dk]
                for dq in range(4):
                    pq = ps_qt.tile([128, 512], f32, tag="pq")
                    for dk in range(4):
                        nc.tensor.matmul(
                            pq,
                            w_sb["wq"][:, dk, dq * 128 : (dq + 1) * 128],
                            xt[:, dk, :],
                            start=(dk == 0),
                            stop=(dk == 3),
                        )
                    nc.vector.tensor_copy(qt_g[dq][:, j * 512 : (j + 1) * 512], pq)

                # all K chains first, then all projection groups: the scalar
                # PSUM->SBUF cast of chunk s completes while later K chunks
                # stream, so the kp matmuls never stall the PE
                e_t = p_e.tile([128, 4, NH, R], cdt, tag="e", name=f"e_{j}")
                nc.sync.dma_start(out=e_t, in_=e_r[:, j, :, :, :])
                k_sbs = []
                for s in range(4):
                    ti = j * 4 + s
                    pk = ps_k.tile([128, 512], f32, tag="pk")
                    for dk in range(4):
                        nc.tensor.matmul(
                            pk,
                            xt[:, dk, s * 128 : (s + 1) * 128],
                            w_sb["wk"][:, dk, :],
                            start=(dk == 0),
                            stop=(dk == 3),
                        )
                    k_sb = p_k.tile([128, 512], cdt, tag="k", name=f"k_{ti}")
                    nc.scalar.copy(k_sb, pk)
                    k_sbs.append(k_sb)

                for s in range(4):
                    ti = j * 4 + s
                    for i in range(4):
                        nc.tensor.matmul(
                            kp_ps[i],
                            k_sbs[s][:, i * 128 : (i + 1) * 128],
                            e_t[:, s, 2 * i : 2 * i + 2, :],
                            start=(ti == 0),
                            stop=(ti == 31),
                        )
            for i in range(4):
                nc.vector.tensor_copy(kp_sb[i], kp_ps[i])

            # ---- second sweep: V, vpT (same pools, rotated buffers) ----
            vp_ps = [
                ps_kp.tile([128, 2 * R], f32, tag=f"acc{i}", name=f"vp_ps{i}")
                for i in range(4)
            ]
            xts[0] = load_x(0, "b")
            for j in range(NT):
                xt = xts.pop(j)
                if j + 1 < NT:
                    xts[j + 1] = load_x(j + 1, "b")

                f_t = p_e.tile([128, 4, NH, R], cdt, tag="e", name=f"f_{j}")
                nc.sync.dma_start(out=f_t, in_=f_r[:, j, :, :, :])
                v_sbs = []
                for s in range(4):
                    ti = j * 4 + s
                    pv = ps_qt.tile([128, 512], f32, tag="pq")
                    for dk in range(4):
                        nc.tensor.matmul(
                            pv,
                            xt[:, dk, s * 128 : (s + 1) * 128],
                            w_sb["wv"][:, dk, :],
                            start=(dk == 0),
                            stop=(dk == 3),
                        )
                    v_sb = p_k.tile([128, 512], cdt, tag="k", name=f"v_{ti}")
                    nc.scalar.copy(v_sb, pv)
                    v_sbs.append(v_sb)

                for s in range(4):
                    ti = j * 4 + s
                    for i in range(4):
                        nc.tensor.matmul(
                            vp_ps[i],
                            v_sbs[s][:, i * 128 : (i + 1) * 128],
                            f_t[:, s, 2 * i : 2 * i + 2, :],
                            start=(ti == 0),
                            stop=(ti == 31),
                        )
            # vext: per pair, cast vpT then transpose that pair's heads to
            # natural layout + ones col, so the PE starts transposing as
            # soon as the first cast retires (ps_k's banks are free once
            # the last V cast retires)
            for i in range(4):
                nc.vector.tensor_copy(vp_sb[i], vp_ps[i])
                for h in (2 * i, 2 * i + 1):
                    off = h % 2
                    pt_f = ps_k.tile([128, 512], f32, tag="pk", name=f"ptf_{h}")
                    pt = pt_f.bitcast(cdt)[:, 0:128]
                    for rc in range(2):
                        nc.tensor.transpose(
                            pt[:, rc * 64 : (rc + 1) * 64],
                            hslice(vp_sb, h)[:, rc * 128 : (rc + 1) * 128],
                            ident[off * 64 : off * 64 + 64, off * 64 : off * 64 + 64],
                        )
                    for rc in range(2):
                        nc.vector.tensor_copy(
                            vext[:, h, rc, 0:64], pt[:, rc * 64 : (rc + 1) * 64]
                        )
                    nc.vector.memset(vext[:, h, :, 64:65], 1.0)

        # ---------------- Phase C: attention + output dense ----------------
        # Software-pipelined: the bc/normalize of head-pair hp issues after
        # the scores/PV of hp+1 (hiding the denominator chain latency), and
        # the output dense of tile j issues inside tile j+1's first block.
        y_r = y_d.rearrange("(j s p) m -> p j s m", s=4, p=128)
        with (
            tc.tile_pool(name="p_at", bufs=6) as p_at,
            tc.tile_pool(name="p_bc", bufs=3) as p_bc,
            tc.tile_pool(name="p_ot", bufs=8) as p_ot,
            tc.tile_pool(name="p_fin", bufs=2) as p_fin,
            # pool order matters: ps_out grabs the banks still held by the
            # A-scope's QT/K pools (PV starts late — plenty of slack) while
            # ps_scfin gets the accumulator banks, which free right after
            # the vp casts — so the first scores never wait on vext
            tc.tile_pool(name="ps_out", bufs=4, space="PSUM") as ps_out,
            tc.tile_pool(name="ps_scfin", bufs=3, space="PSUM") as ps_scfin,
            tc.tile_pool(name="ps_bc", bufs=1, space="PSUM") as ps_bc,
        ):
            nc.sync.dma_start(
                out=w_sb["wo"], in_=wo_d.rearrange("(dk p) m -> p dk m", p=128)
            )
            b_bc_ap = bass.AP(
                tensor=b_d.tensor, offset=b_d.offset, ap=[[0, 128]] + list(b_d.ap)
            )
            nc.sync.dma_start(out=bias_bc128, in_=b_bc_ap)
            nc.vector.tensor_copy(bias_bf, bias_bc128[0:1, :])

            pend = None  # (oT list, hp, outps) awaiting bc+normalize
            pend_fin = None  # oT list awaiting output dense

            def attn_block(j, hp, oT):
                """scores + exp for both heads, then PV for both heads —
                each PV's exp input gets >=2 matmuls of cover, so the PE
                never waits on the scalar engine. The previous pair's bc
                flush is issued AFTER this block: its den rows then beat
                the recip/muls into the Vector queue, and the bc matmul
                gets a full block of PE cover."""
                den = den_bf[hp % 2]
                ats, outps = [], []
                for hh in range(2):
                    h = hp * 2 + hh
                    qrow = qt_g[h // 2][
                        (h % 2) * 64 : (h % 2) * 64 + 64, j * 512 : (j + 1) * 512
                    ]
                    at = []
                    for rc in range(2):
                        sc = ps_scfin.tile([128, 512], f32, tag="sf")
                        nc.tensor.matmul(
                            sc,
                            hslice(kp_sb, h)[:, rc * 128 : (rc + 1) * 128],
                            qrow,
                            start=True,
                            stop=True,
                        )
                        a = p_at.tile([128, 512], cdt, tag="at")
                        nc.scalar.activation(
                            a, sc, mybir.ActivationFunctionType.Exp, scale=0.125
                        )
                        at.append(a)
                    ats.append(at)
                for hh in range(2):
                    h = hp * 2 + hh
                    # PV with ones column: rows 0..63 = outT, row 64 = denom
                    op = ps_out.tile([128, 512], f32, tag="op")
                    for rc in range(2):
                        nc.tensor.matmul(
                            op[0:65, :],
                            vext[:, h, rc, :],
                            ats[hh][rc],
                            start=(rc == 0),
                            stop=(rc == 1),
                        )
                    outps.append(op)
                    if hh == 0:
                        nc.scalar.copy(den[0:1, :], op[64:65, :])
                    else:
                        nc.vector.tensor_copy(den[64:65, :], op[64:65, :])
                return outps

            def flush_bc(oT, hp, outps):
                """selector-broadcast raw denoms, reciprocal, normalize."""
                bc = ps_bc.tile([128, 512], f32, tag="bc")
                nc.tensor.matmul(bc, sel65, den_bf[hp % 2], start=True, stop=True)
                bc_sb = p_bc.tile([128, 512], f32, tag="bcs")
                nc.vector.reciprocal_approx_fast(out=bc_sb, in_=bc)
                for hh in range(2):
                    nc.vector.tensor_mul(
                        oT[hp][hh * 64 : (hh + 1) * 64, :],
                        outps[hh][0:64, :],
                        bc_sb[hh * 64 : (hh + 1) * 64, :],
                    )

            def fin_chunk(fin, j, oT, s, fp=None):
                """one 128-row dense chunk. Even s: rank-1 bias matmul +
                scalar copy; odd s: bias folded into the vector transfer
                as a tensor_add (same cost as the copy it replaces)."""
                odd = s % 2 == 1
                first = 3 if fp is not None else 0
                if fp is None:
                    fp = ps_scfin.tile([128, 512], f32, tag="sf")
                for dm in range(first, 4):
                    nc.tensor.matmul(
                        fp,
                        oT[dm][:, s * 128 : (s + 1) * 128],
                        w_sb["wo"][:, dm, :],
                        start=(dm == 0),
                        stop=(odd and dm == 3),
                    )
                if odd:
                    nc.vector.tensor_add(fin[:, s, :], fp, bias_bc128)
                else:
                    nc.tensor.matmul(fp, ones_col, bias_bf, start=False, stop=True)
                    nc.scalar.copy(fin[:, s, :], fp)
                nc.gpsimd.dma_start(out=y_r[:, j, s, :], in_=fin[:, s, :])

            def flush_fin(j, oT):
                """output dense + bias for tile j (issued during j+1)."""
                fin = p_fin.tile([128, 4, 512], f32, tag="fin", name=f"fin_{j}")
                for s in range(4):
                    fin_chunk(fin, j, oT, s)

            for j in range(NT):
                oT = [
                    p_ot.tile([128, 512], cdt, tag="ot", name=f"oT{j}_{t}")
                    for t in range(4)
                ]
                for hp in range(4):
                    outps = attn_block(j, hp, oT)
                    if pend is not None:
                        flush_bc(*pend)
                    if hp == 0 and pend_fin is not None:
                        flush_fin(j - 1, pend_fin)
                        pend_fin = None
                    pend = (oT, hp, outps)
                pend_fin = oT
            # last tile: overlap the final pair's normalize chain with
            # partial dense chains (oT[dm] only depends on head pair dm,
            # so dm=0..2 can accumulate before hp3's normalize lands)
            oT = pend_fin
            fin = p_fin.tile([128, 4, 512], f32, tag="fin", name="fin_last")
            fps = []
            for s in range(3):
                fp = ps_scfin.tile([128, 512], f32, tag="sf")
                for dm in range(3):
                    nc.tensor.matmul(
                        fp,
                        oT[dm][:, s * 128 : (s + 1) * 128],
                        w_sb["wo"][:, dm, :],
                        start=(dm == 0),
                        stop=False,
                    )
                fps.append(fp)
                if s == 0:
                    flush_bc(*pend)
            for s in range(3):
                fin_chunk(fin, NT - 1, oT, s, fp=fps[s])
            fin_chunk(fin, NT - 1, oT, 3)

    nc.compile()
    _built["nc"] = nc
    return nc


def _runner():
    """Build (once) a cached jitted 8-core executor for the Bass module."""
    if "run" in _built:
        return _built["run"]

    import jax
    import numpy as _np

    import concourse.mybir as mybir
    from concourse import bass2jax

    bass2jax.install_neuronx_cc_hook()
    nc = _build()

    part_name = nc.partition_id_tensor.name if nc.partition_id_tensor else None
    in_names, out_names, out_avals = [], [], []
    for alloc in nc.m.functions[0].allocations:
        if not isinstance(alloc, mybir.MemoryLocationSet):
            continue
        name = alloc.memorylocations[0].name
        if alloc.kind == "ExternalInput":
            if name != part_name:
                in_names.append(name)
        elif alloc.kind == "ExternalOutput":
            out_names.append(name)
            out_avals.append(
                jax.core.ShapedArray(
                    tuple(alloc.tensor_shape), mybir.dt.np(alloc.dtype)
                )
            )
    n_outs = len(out_avals)
    all_in_names = tuple(
        in_names + out_names + ([part_name] if part_name else [])
    )

    from jax.sharding import NamedSharding

    def _body(*args):
        operands = list(args)
        if part_name is not None:
            operands.append(bass2jax.partition_id_tensor())
        outs = bass2jax._bass_exec_p.bind(
            *operands,
            out_avals=tuple(out_avals),
            in_names=all_in_names,
            out_names=tuple(out_names),
            lowering_input_output_aliases=(),
            sim_require_finite=True,
            sim_require_nnan=True,
            nc=nc,
        )
        return tuple(outs)

    devices = jax.devices()[:NCORES]
    mesh = bass2jax.Mesh(_np.asarray(devices), ("core",))
    p_core = bass2jax.PartitionSpec("core")
    p_repl = bass2jax.PartitionSpec()
    in_specs = tuple(p_core if n == "x" else p_repl for n in in_names) + (
        p_core,
    ) * n_outs
    sharded = jax.jit(
        bass2jax.shard_map(
            _body,
            mesh=mesh,
            in_specs=in_specs,
            out_specs=(p_core,) * n_outs,
            check_rep=False,
        ),
        keep_unused=True,
    )
    sh_core = NamedSharding(mesh, p_core)
    sh_repl = NamedSharding(mesh, p_repl)
    dev_cache = {}
    zero_cache = {}

    def run(in_maps):
        args = []
        for name in in_names:
            if name == "x":
                xc = np.concatenate([np.asarray(m[name]) for m in in_maps], axis=0)
                args.append(jax.device_put(xc, sh_core))
            else:
                a = np.asarray(in_maps[0][name])
                key = (name, a.shape, str(a.dtype), hash(a.tobytes()))
                if key not in dev_cache:
                    dev_cache.clear() if len(dev_cache) > 64 else None
                    dev_cache[key] = jax.device_put(a, sh_repl)
                args.append(dev_cache[key])
        for i, a in enumerate(out_avals):
            if i not in zero_cache:
                zero_cache[i] = jax.device_put(
                    np.zeros((NCORES * a.shape[0], *a.shape[1:]), a.dtype), sh_core
                )
            args.append(zero_cache[i])
        out_arrs = sharded(*args)
        return [
            {
                name: np.asarray(out_arrs[i]).reshape(
                    NCORES, *out_avals[i].shape
                )[c]
                for i, name in enumerate(out_names)
            }
            for c in range(NCORES)
        ]

    _built["run"] = run
    return run


def kernel(x, wq, wk, wv, E, F, w_out, b_out):
    """Full inputs in, full output out. Shards batch across 8 cores."""
    run = _runner()

    np_c = ml_dtypes.bfloat16
    wq_c = np.ascontiguousarray(wq, dtype=np_c)
    wk_c = np.ascontiguousarray(wk, dtype=np_c)
    wv_c = np.ascontiguousarray(wv, dtype=np_c)
    e_c = prep_ef(E)
    f_c = prep_ef(F)
    wo_c = np.ascontiguousarray(w_out, dtype=np_c)
    b_c = np.ascontiguousarray(b_out, dtype=np.float32)

    in_maps = [
        {
            "x": prep_x(x[i]),
            "wq": wq_c,
            "wk": wk_c,
            "wv": wv_c,
            "E": e_c,
            "F": f_c,
            "w_out": wo_c,
            "b_out": b_c,
        }
        for i in range(NCORES)
    ]
    results = run(in_maps)
    return np.stack([results[i]["y"] for i in range(NCORES)], axis=0)


if __name__ == "__main__":
    xs = {
        "x": np.random.randn(BATCH, SEQ, DM).astype(np.float32),
        "wq": np.random.randn(DM, DM).astype(np.float32) * 0.05,
        "wk": np.random.randn(DM, DM).astype(np.float32) * 0.05,
        "wv": np.random.randn(DM, DM).astype(np.float32) * 0.05,
        "E": np.random.randn(NH, SEQ, R).astype(np.float32) * 0.03,
        "F": np.random.randn(NH, SEQ, R).astype(np.float32) * 0.03,
        "w_out": np.random.randn(DM, DM).astype(np.float32) * 0.05,
        "b_out": np.zeros(DM, np.float32),
    }
    y = kernel(**xs)
    print(y.shape, y.dtype)
